# revision 18
# baseline (speedup 1.0000x reference)
"""Trainium2 Bass kernel for nn_CrossAttentionLayer (sparse cross attention).

Sharding: 8 cores = 4 batches x 2 head-groups. Core c handles batch c//2 and
heads [4*(c%2), 4*(c%2)+4). Host compacts the key side by kv_mask (~50% of
keys are masked out for the whole batch), so each core only computes
attention over KP = roundup(max_valid_keys, 128) keys instead of 2048.
kv_mask is folded into the shipped sparse mask (padding columns are zero),
so no separate kv-mask multiply is needed on device.

Device algorithm (per core), matmuls bf16 with fp32 PSUM accumulation:
  xlnT   = dma-transpose(layernorm(x))        (LN gains/biases folded into W/b)
  qT/kT  = W.T @ xlnT   [d, tok]              (per-partition bias add on DVE)
  v      = xlnT.T @ Wv  [tok, d]              (+ ones column for denominator)
  sT     = kT.T-blocks @ qT-blocks            [k, q] scores, transposed
  pT     = exp(sT * scale)                    (ACT) * sparse_maskT (DVE/Pool)
  accT   = [v | 1].T @ pT                     rows 0-63 = unnormalized out.T,
                                              row 64 = softmax denominator
  den    = Copy(accT[64] + 1e-30)             (ACT, bf16)
  rb     = ones.T @ den                       (PE broadcast to 64 partitions)
  aT     = accT[0:64] / rb                    (DVE divide)
  out    = aT.T-blocks @ Wo-blocks            [q, E] partial, fp32 to HBM

Engine placement (GPSIMD cannot touch PSUM): exp + den-copies on ACT;
part of mask-mult + bn_stats + all PSUM->SBUF copies + divide on DVE;
LN sig (pow) + LN-apply + the rest of mask-mult on Pool (SBUF-only);
transposes on the DMA xbar; matmuls on PE.
Host sums the two per-batch partials and adds bo.
"""

import os

import numpy as np
import ml_dtypes

import bass_rust
import concourse.bass as bass
import concourse.mybir as mybir
import concourse.tile as tile
from concourse import bass_utils
from concourse.masks import make_identity
from concourse.vector_clock import ScopedClock


class _TileContext(tile.TileContext):
    """TileContext whose kernel-tail drain is split into single-wait drains.

    The walrus build in this environment rejects >1 sync-wait on a Drain
    (CTRL_NO struct): "Too many sync wait commands". The stock
    _drain_and_barrier attaches one wait per outstanding semaphore to a
    single Drain; emit one Drain per wait instead.
    """

    def _drain_and_barrier(self, tick_clock, wait_clock):
        drain_inst = self.nc.sync.drain()
        wait_clock.add_sem_waits(
            drain_inst.ins, ScopedClock({None: tick_clock.global_clock})
        )
        si = drain_inst.ins.sync_info
        if si is not None and si.on_wait and len(si.on_wait) > 1:
            waits = list(si.on_wait)
            drain_inst.ins.sync_info = bass_rust.SyncInfo(
                on_wait=[waits[0]], on_update=si.on_update or [])
            for w in waits[1:]:
                extra = self.nc.sync.drain()
                extra.ins.sync_info = bass_rust.SyncInfo(
                    on_wait=[w], on_update=[])

        self.nc.all_engine_barrier()
        assert self.sems is not None
        popped = self.nc._tile_sem_poison_stack.pop()
        assert popped is self._sem_poison
        self.nc.clear_and_free_semaphores(list(self.sems.allocated().values()))
        self.nc.all_engine_barrier()

def _split_sync_waits(nc):
    """Cap every instruction at one sync wait.

    This walrus build rejects instructions carrying more than one sem wait
    ("Too many sync wait commands", setupSyncWait) across several structs
    (Drain, DMACopy, ...). Move excess waits onto no-op instructions placed
    immediately before the offender on the same engine — identical ordering
    semantics, one wait per instruction.
    """
    for f in nc.m.functions:
        for bb in f.blocks:
            insns = bb.instructions
            out = []
            changed = False
            for ins in insns:
                si = ins.sync_info
                if si is not None and si.on_wait and len(si.on_wait) > 1:
                    waits = list(si.on_wait)
                    for w in waits[:-1]:
                        nop = mybir.InstNoOp(
                            name=nc.get_next_instruction_name(),
                            engine=ins.engine,
                            ins=[], outs=[],
                            sync_info=bass_rust.SyncInfo(
                                on_wait=[w], on_update=[]),
                        )
                        out.append(nop)
                    ins.sync_info = bass_rust.SyncInfo(
                        on_wait=[waits[-1]], on_update=si.on_update or [])
                    changed = True
                out.append(ins)
            if changed:
                bb.instructions = out


BF16 = ml_dtypes.bfloat16

E = 512
H = 8
D = 64
T = 2048           # query tokens
P = 128
NT = T // P        # 16 query token tiles
EC = E // P        # 4 contraction chunks
HC = 4             # heads per core
MC = 2             # 128-wide chunks of this core's 256 head dims
QC = 4             # 512-wide query chunks
SCALE = float(D) ** -0.5
EPS = 1e-5

_CACHE = {}


def _build(needs_bv: bool, ntk: int, reps: int = 1):
    """ntk = number of 128-token key chunks after host-side compaction."""
    nc = bass.Bass("TRN2", target_bir_lowering=False, debug=False, num_devices=8)
    f32 = mybir.dt.float32
    bf16 = mybir.dt.bfloat16
    KT = ntk * P

    # kv tile groups of up to 4 tiles (512 tokens)
    kvg = [list(range(g, min(g + 4, ntk))) for g in range(0, ntk, 4)]
    NKG = len(kvg)

    xq = nc.dram_tensor("xq", [T, E], bf16, kind="ExternalInput").ap()
    xkv = nc.dram_tensor("xkv", [KT, E], bf16, kind="ExternalInput").ap()
    wq = nc.dram_tensor("wq", [E, MC * P], bf16, kind="ExternalInput").ap()
    wk = nc.dram_tensor("wk", [E, MC * P], bf16, kind="ExternalInput").ap()
    wv = nc.dram_tensor("wv", [E, MC * P], bf16, kind="ExternalInput").ap()
    wo = nc.dram_tensor("wo", [MC * P, E], bf16, kind="ExternalInput").ap()
    bqd = nc.dram_tensor("bq", [P, MC], f32, kind="ExternalInput").ap()
    bkd = nc.dram_tensor("bk", [P, MC], f32, kind="ExternalInput").ap()
    mtd = nc.dram_tensor("mt", [KT, T], bf16, kind="ExternalInput").ap()
    if needs_bv:
        bvd = nc.dram_tensor("bv", [1, MC * P], bf16, kind="ExternalInput").ap()
    outd = nc.dram_tensor("out", [T, E], bf16, kind="ExternalOutput").ap()

    with _TileContext(nc) as tc:
        with (
            tc.tile_pool(name="persist", bufs=1) as pp,
            tc.tile_pool(name="work", bufs=5) as wk_pool,
            tc.tile_pool(name="scratch", bufs=4) as scratch,
            tc.tile_pool(name="psA", bufs=2, space="PSUM") as psA,
            tc.tile_pool(name="psS", bufs=2, space="PSUM") as psS,
            tc.tile_pool(name="psC", bufs=2, space="PSUM") as psC,
        ):
            # ---- persistent SBUF tensors ----
            wq_sb = pp.tile([P, EC, MC * P], bf16, tag="wq")
            wk_sb = pp.tile([P, EC, MC * P], bf16, tag="wk")
            wv_sb = pp.tile([P, EC, MC * P], bf16, tag="wv")
            wo_sb = pp.tile([P, MC, E], bf16, tag="wo")
            bq_sb = pp.tile([P, MC], f32, tag="bq")
            bk_sb = pp.tile([P, MC], f32, tag="bk")
            mt_sb = pp.tile([P, ntk, T], bf16, tag="mt")
            xlnq_g = [pp.tile([P, 4, EC, P], bf16, tag=f"xlnq{g}",
                              name=f"xlnq{g}") for g in range(4)]
            xlnkv_g = [pp.tile([P, len(kvg[g]), EC, P], bf16, tag=f"xlnkv{g}",
                               name=f"xlnkv{g}") for g in range(NKG)]
            qT_g = [pp.tile([P, MC, 512], bf16, tag=f"qT{g}", name=f"qT{g}")
                    for g in range(4)]
            kT_gt = [pp.tile([P, MC, len(kvg[g]) * P], bf16, tag=f"kT{g}",
                             name=f"kT{g}") for g in range(NKG)]
            v_gt = [pp.tile([P, len(kvg[g]), HC * (D + 1)], bf16, tag=f"v{g}",
                            name=f"v{g}") for g in range(NKG)]
            aT_g = [pp.tile([P, MC, 512], bf16, tag=f"aT{g}", name=f"aT{g}")
                    for g in range(4)]
            xall_q = pp.tile([P, NT, E], bf16, tag="xallq")
            xall_kv = pp.tile([P, ntk, E], bf16, tag="xallkv")
            if needs_bv:
                bv_sb = pp.tile([1, MC * P], bf16, tag="bv")
                ones_sb = pp.tile([1, P], bf16, tag="ones")

            if needs_bv:
                nc.sync.dma_start(bv_sb[:], bvd)
                nc.vector.memset(ones_sb[:], 1.0)
            ones1 = pp.tile([P, D], bf16, tag="ones1")
            nc.vector.memset(ones1[:], 1.0)
            eps_sb = pp.tile([P, 1], f32, tag="eps")
            nc.vector.memset(eps_sb[:], EPS)
            ident = pp.tile([P, P], bf16, tag="ident")
            make_identity(nc, ident[:])
            mtr = mtd.rearrange("(c p) q -> p c q", p=P)
            xqr = xq.rearrange("(t p) e -> p t e", p=P)
            xkvr = xkv.rearrange("(t p) e -> p t e", p=P)

            def ln_group(xall, xslots, dstT):
                """LN resident token tiles xslots into dstT[:, i].

                Emitted in two batched phases (all stats, then all applies)
                so no engine's in-order SEQ ever waits on a cross-engine
                result that was emitted fewer than ~4 instructions earlier.
                """
                mvs = []
                for xslot in xslots:
                    stats = scratch.tile([P, 6], f32, tag="bnstats")
                    mv = scratch.tile([P, 2], f32, tag="bnmv")
                    nc.vector.bn_stats(stats[:], xall[:, xslot])
                    nc.vector.bn_aggr(mv[:], stats[:])
                    mvs.append(mv)
                sigs = []
                for mv in mvs:
                    sig = scratch.tile([P, 1], f32, tag="sig")
                    nc.scalar.activation(
                        sig[:], mv[:, 1:2],
                        mybir.ActivationFunctionType.Sqrt, bias=eps_sb[:])
                    sigs.append(sig)
                rsigs = []
                for sig in sigs:
                    rsig = scratch.tile([P, 1], f32, tag="rsig")
                    nc.vector.reciprocal(rsig[:], sig[:])
                    rsigs.append(rsig)
                for i, xslot in enumerate(xslots):
                    xln = wk_pool.tile([P, E], bf16, tag="xln")
                    nc.vector.tensor_scalar(
                        xln[:], xall[:, xslot], mvs[i][:, 0:1], rsigs[i][:],
                        mybir.AluOpType.subtract, mybir.AluOpType.mult)
                    # blocked transpose on the DMA xbar: dstT[p, c, tok]
                    nc.sync.dma_start_transpose(dstT[:, i], xln[:])

            def kproj_group(g):
                L = len(kvg[g]) * P
                for mc in range(MC):
                    ps = psA.tile([P, 512], mybir.dt.float32, tag="p512")
                    for c in range(EC):
                        nc.tensor.matmul(
                            ps[:, 0:L],
                            lhsT=wk_sb[:, c, mc * P:(mc + 1) * P],
                            rhs=xlnkv_g[g][:, :, c, :],
                            start=(c == 0), stop=(c == EC - 1))
                    nc.vector.tensor_scalar(
                        kT_gt[g][:, mc, :], ps[:, 0:L], bk_sb[:, mc:mc + 1],
                        None, mybir.AluOpType.add)

            def vproj_group(g):
                vr = v_gt[g].rearrange("p u (h d) -> p u h d", d=D + 1)
                nc.gpsimd.memset(vr[:, :, :, D], 1.0)
                for ti in range(len(kvg[g])):
                    ps = psA.tile([P, MC * P], mybir.dt.float32, tag="p512")
                    for c in range(EC):
                        nc.tensor.matmul(
                            ps[:],
                            lhsT=xlnkv_g[g][:, ti, c, :],
                            rhs=wv_sb[:, c, :],
                            start=(c == 0),
                            stop=(c == EC - 1 and not needs_bv))
                    if needs_bv:
                        nc.tensor.matmul(
                            ps[:], lhsT=ones_sb[:], rhs=bv_sb[:],
                            start=False, stop=True)
                    nc.vector.tensor_copy(
                        vr[:, ti, :, 0:D],
                        ps.rearrange("p (h d) -> p h d", d=D))

            def qproj_group(g):
                for mc in range(MC):
                    ps = psA.tile([P, 512], mybir.dt.float32, tag="p512")
                    for c in range(EC):
                        nc.tensor.matmul(
                            ps[:],
                            lhsT=wq_sb[:, c, mc * P:(mc + 1) * P],
                            rhs=xlnq_g[g][:, :, c, :],
                            start=(c == 0), stop=(c == EC - 1))
                    nc.vector.tensor_scalar(
                        qT_g[g][:, mc, :], ps[:], bq_sb[:, mc:mc + 1],
                        None, mybir.AluOpType.add)

            # Prefetch everything up front: no DMA issued later ever blocks
            # the in-order SP queue behind a data-dependent transpose wait.
            rep_ctx = tc.For_i(0, reps, 1) if reps > 1 else None
            if rep_ctx is not None:
                rep_ctx.__enter__()
            for g in range(NKG):
                nc.sync.dma_start(
                    xall_kv[:, 4 * g:4 * g + len(kvg[g])],
                    xkvr[:, 4 * g:4 * g + len(kvg[g])])
            nc.sync.dma_start(
                wk_sb[:], wk.rearrange("(c p) n -> p c n", p=P))
            nc.sync.dma_start(
                wv_sb[:], wv.rearrange("(c p) n -> p c n", p=P))
            nc.sync.dma_start(bk_sb[:], bkd)
            nc.sync.dma_start(
                xall_q[:, 0:4], xqr[:, 0:4])
            nc.sync.dma_start(
                wq_sb[:], wq.rearrange("(c p) n -> p c n", p=P))
            nc.sync.dma_start(bq_sb[:], bqd)
            nc.sync.dma_start(
                wo_sb[:], wo.rearrange("(c p) n -> p c n", p=P))
            for c in range(ntk):
                nc.sync.dma_start(mt_sb[:, c], mtr[:, c])
            for g in range(1, 4):
                nc.sync.dma_start(
                    xall_q[:, 4 * g:4 * g + 4], xqr[:, 4 * g:4 * g + 4])

            # kv side first (attention needs all of kT/v); q-side groups
            # unblock attention per qc.
            for g in range(NKG):
                ln_group(xall_kv, kvg[g], xlnkv_g[g])
                kproj_group(g)
                vproj_group(g)
                if g == 1:
                    # q group 0 early: attention (qc=0) starts on k groups
                    # 0-1 while the kv tail is still in layernorm
                    ln_group(xall_q, list(range(4)), xlnq_g[0])
                    qproj_group(0)
            if NKG < 2:
                ln_group(xall_q, list(range(4)), xlnq_g[0])
                qproj_group(0)
            for g in range(1, 4):
                ln_group(xall_q, list(range(4 * g, 4 * g + 4)), xlnq_g[g])
                qproj_group(g)

            # ---- attention: qc outer so normalize+out_proj overlap ----
            # The per-head normalize epilogue is emitted DEFERRED, inside the
            # next head's pass stream, so its sem waits are already satisfied
            # when each engine's in-order SEQ reaches them (no head-of-line
            # blocking of the exp/mask/matmul streams).
            npass = (ntk + 1) // 2

            def flush_norm(pend):
                qc, h, acc = pend
                mc = h // 2
                po = (h % 2) * D
                # denominator: row D of acc -> bf16 row, PE-broadcast to
                # 64 partitions, back to SBUF, then divide
                rs = wk_pool.tile([P, 512], bf16, tag="rs")
                nc.vector.tensor_copy(rs[D:D + 1, :], acc[D:D + 1, :])
                rb = psA.tile([P, 512], mybir.dt.float32, tag="p512")
                nc.tensor.matmul(
                    rb[0:D], lhsT=ones1[D:D + 1, :], rhs=rs[D:D + 1, :],
                    start=True, stop=True)
                rbs = wk_pool.tile([P, 512], f32, tag="rbs")
                nc.vector.reciprocal(rbs[0:D, :], rb[0:D, :])
                nc.vector.tensor_tensor(
                    aT_g[qc][po:po + D, mc, :], acc[0:D], rbs[0:D, :],
                    mybir.AluOpType.mult)

            pend = None
            for qc in range(QC):
                for h in range(HC):
                    mc = h // 2
                    po = (h % 2) * D
                    acc = psC.tile([P, 512], mybir.dt.float32, tag="acc")
                    for kcp in range(npass):
                        w = 2 if 2 * kcp + 1 < ntk else 1
                        sp = psS.tile([P, 2, 512], mybir.dt.float32, tag="sp")
                        for j in range(w):
                            ci = 2 * kcp + j
                            kg, ko = ci // 4, ci % 4
                            nc.tensor.matmul(
                                sp[:, j],
                                lhsT=kT_gt[kg][po:po + D, mc,
                                               ko * P:(ko + 1) * P],
                                rhs=qT_g[qc][po:po + D, mc, :],
                                start=True, stop=(w == 2))
                        if w == 1:
                            # odd tail chunk: add -200*(1-mask) into the
                            # scores via the PE (host ships that chunk of mt
                            # pre-transformed); exp then masks for free
                            nc.tensor.matmul(
                                sp[:, 0], lhsT=ident[:],
                                rhs=mt_sb[:, ntk - 1,
                                          qc * 512:(qc + 1) * 512],
                                start=False, stop=True)
                        pT = wk_pool.tile([P, 2, 512], bf16, tag="pT")
                        nc.scalar.activation(
                            pT[:, 0:w], sp[:, 0:w],
                            mybir.ActivationFunctionType.Exp,
                            scale=SCALE)
                        if w == 2:
                            # sparse-mask multiply: split pairs across DVE
                            # and the otherwise-idle Pool engine
                            mask_eng = nc.vector if kcp % 2 == 0 else nc.gpsimd
                            mask_eng.tensor_tensor(
                                pT[:, 0:w], pT[:, 0:w],
                                mt_sb[:, 2 * kcp:2 * kcp + w,
                                      qc * 512:(qc + 1) * 512],
                                mybir.AluOpType.mult)
                        if kcp == 1 and pend is not None:
                            flush_norm(pend)
                            pend = None
                        for j in range(w):
                            ci = 2 * kcp + j
                            kg, ko = ci // 4, ci % 4
                            nc.tensor.matmul(
                                acc[:D + 1],
                                lhsT=v_gt[kg][:, ko,
                                              h * (D + 1):(h + 1) * (D + 1)],
                                rhs=pT[:, j],
                                start=(ci == 0),
                                stop=(ci == ntk - 1))
                    pend = (qc, h, acc)

                # out projection for this qc's 4 token tiles
                flush_norm(pend)
                pend = None
                for ti in range(4):
                    t = qc * 4 + ti
                    ps = psA.tile([P, E], mybir.dt.float32, tag="p512")
                    for mc in range(MC):
                        nc.tensor.matmul(
                            ps[:],
                            lhsT=aT_g[qc][:, mc, ti * P:(ti + 1) * P],
                            rhs=wo_sb[:, mc, :],
                            start=(mc == 0), stop=(mc == MC - 1))
                    osb = wk_pool.tile([P, E], bf16, tag="osb")
                    nc.vector.tensor_copy(osb[:], ps[:])
                    nc.sync.dma_start(outd[t * P:(t + 1) * P, :], osb[:])

            if rep_ctx is not None:
                rep_ctx.__exit__(None, None, None)

    _split_sync_waits(nc)
    return nc


def _get_nc(needs_bv: bool, ntk: int = 9, reps: int = 1):
    key = ("nc", needs_bv, ntk, reps)
    if key not in _CACHE:
        _CACHE[key] = _build(needs_bv, ntk, reps)
    return _CACHE[key]


def kernel(query, key_value, kv_mask, sparse_mask,
           ln_q_g, ln_q_b, ln_kv_g, ln_kv_b,
           Wq, bq, Wk, bk, Wv, bv, Wo, bo):
    query = np.asarray(query, np.float32)
    key_value = np.asarray(key_value, np.float32)
    kv_mask = np.asarray(kv_mask)
    sparse_mask = np.asarray(sparse_mask)
    B = query.shape[0]

    # Fold LN gain/bias into the projection weights (exact algebra):
    # (x_ln*g + b) @ W + c  ==  x_ln @ (g[:,None]*W) + (b@W + c)
    Wq_g = np.asarray(ln_q_g, np.float32)[:, None] * np.asarray(Wq, np.float32)
    Wk_g = np.asarray(ln_kv_g, np.float32)[:, None] * np.asarray(Wk, np.float32)
    Wv_g = np.asarray(ln_kv_g, np.float32)[:, None] * np.asarray(Wv, np.float32)
    bq_e = np.asarray(ln_q_b, np.float32) @ np.asarray(Wq, np.float32) + bq
    bk_e = np.asarray(ln_kv_b, np.float32) @ np.asarray(Wk, np.float32) + bk
    bv_e = np.asarray(ln_kv_b, np.float32) @ np.asarray(Wv, np.float32) + bv

    # Key compaction: only keys with kv_mask=1 participate anywhere.
    counts = kv_mask.sum(axis=1)
    KP = max(P, int(-(-int(counts.max()) // P)) * P)
    NTK = KP // P

    needs_bv = bool(np.any(bv_e != 0.0))
    reps = int(os.environ.get("KERNEL_REPS", "1"))
    nc = _get_nc(needs_bv, NTK, reps)

    xkv_b, mt_b = [], []
    for b in range(B):
        idx = np.flatnonzero(kv_mask[b])
        xkv_c = np.zeros((KP, E), dtype=BF16)
        xkv_c[:len(idx)] = key_value[b][idx].astype(BF16)
        mt_f = np.zeros((KP, T), np.float32)
        mt_f[:len(idx)] = sparse_mask[b].T[idx]
        if NTK % 2 == 1:
            # odd tail chunk is applied as an additive -200*(1-m) bias via
            # the PE (see _build); ship it pre-transformed
            tail = slice((NTK - 1) * P, NTK * P)
            mt_f[tail] = -200.0 * (1.0 - mt_f[tail])
        xkv_b.append(xkv_c)
        mt_b.append(mt_f.astype(BF16))

    in_maps = []
    for c in range(8):
        b, hg = c // 2, c % 2
        hs = slice(hg * MC * P, (hg + 1) * MC * P)
        m = {
            "xq": np.ascontiguousarray(query[b]).astype(BF16),
            "xkv": xkv_b[b],
            "wq": np.ascontiguousarray(Wq_g[:, hs]).astype(BF16),
            "wk": np.ascontiguousarray(Wk_g[:, hs]).astype(BF16),
            "wv": np.ascontiguousarray(Wv_g[:, hs]).astype(BF16),
            "wo": np.ascontiguousarray(np.asarray(Wo, np.float32)[hs, :]).astype(BF16),
            "bq": np.ascontiguousarray(bq_e[hs].reshape(MC, P).T),
            "bk": np.ascontiguousarray(bk_e[hs].reshape(MC, P).T),
            "mt": mt_b[b],
        }
        if needs_bv:
            m["bv"] = bv_e[hs].astype(BF16).reshape(1, MC * P)
        in_maps.append(m)

    res = bass_utils.run_bass_kernel_spmd(
        nc, in_maps, core_ids=list(range(8)),
        trace=bool(os.environ.get("KERNEL_TRACE")))
    globals()["LAST_RESULTS"] = res

    bo_f = np.asarray(bo, np.float32)
    out = np.empty((B, T, E), np.float32)
    for b in range(B):
        out[b] = (res.results[2 * b]["out"].astype(np.float32)
                  + res.results[2 * b + 1]["out"].astype(np.float32) + bo_f)
    return out
